# revision 3
# baseline (speedup 1.0000x reference)
"""FP6Linear (fake-quant-dequant weight + linear) on 8 Trainium2 NeuronCores.

Strategy: column-parallel tensor parallelism. Each core gets a 2048-row shard
of W (out_features) and bias, with x replicated. Inputs are staged K-major
(transposed on host) so both matmul operands load contiguously with the
contraction dim on partitions. The FP6 fake-quant-dequant runs on device:
per-core abs-max reduce, a tiny AllReduce(max) collective for the global
scale, then an exact round-to-nearest-even dequant chain into a bf16 W.T
cache held fully in SBUF. Matmuls run in bf16 with fp32 PSUM accumulation.
"""

import numpy as np

import concourse.bacc as bacc
import concourse.bass as bass
import concourse.bass_isa as bass_isa
import concourse.mybir as mybir
import concourse.tile as tile
from concourse import bass_utils

# Problem shapes (hardcoded per contract)
B, S, D_IN, D_OUT = 4, 2048, 4096, 16384
M = B * S               # 8192 rows of x
K = D_IN                # 4096 contraction
N_CORES = 8
N = D_OUT // N_CORES    # 2048 out-features per core
P = 128
KB = K // P             # 32 k-blocks
MT = M // P             # 64 m-tiles
NQ = 4                  # psum n-chunks per m-tile
NQS = N // NQ           # 512

FP32 = mybir.dt.float32
BF16 = mybir.dt.bfloat16

_COMPILED = {}


def _build():
    nc = bacc.Bacc(
        "TRN2",
        target_bir_lowering=False,
        debug=False,
        enable_asserts=False,
        num_devices=N_CORES,
    )
    xT_d = nc.dram_tensor("xT", [K, M], FP32, kind="ExternalInput").ap()
    wT_d = nc.dram_tensor("wT", [K, N], FP32, kind="ExternalInput").ap()
    bias_d = nc.dram_tensor("bias", [1, N], FP32, kind="ExternalInput").ap()
    y_d = nc.dram_tensor("y", [M, N], FP32, kind="ExternalOutput").ap()

    with tile.TileContext(nc) as tc:
        with (
            tc.tile_pool(name="const", bufs=1) as const,
            tc.tile_pool(name="wt", bufs=1) as wt_pool,
            tc.tile_pool(name="big", bufs=2) as big,
            tc.tile_pool(name="xload", bufs=2) as xload,
            tc.tile_pool(name="xt", bufs=2) as xt_pool,
            tc.tile_pool(name="psum", bufs=2, space="PSUM") as psum,
            tc.tile_pool(name="dram", bufs=1, space="DRAM") as dram,
        ):
            # ---- constants ----
            bias_rep = const.tile([P, N], FP32)
            nc.sync.dma_start(bias_rep[:], bias_d.to_broadcast((P, N)))

            # ---- pass 1: local |W| max ----
            amax = const.tile([P, KB], FP32)
            for kb in range(KB):
                wl = big.tile([P, N], FP32, tag="b2k")
                nc.sync.dma_start(wl[:], wT_d[kb * P : (kb + 1) * P, :])
                nc.vector.tensor_reduce(
                    amax[:, kb : kb + 1], wl[:], mybir.AxisListType.X,
                    mybir.AluOpType.max, apply_absolute_value=True,
                )
            amax1 = const.tile([P, 1], FP32)
            nc.vector.tensor_reduce(
                amax1[:], amax[:], mybir.AxisListType.X, mybir.AluOpType.max
            )
            amax_pr = const.tile([P, 1], FP32)
            nc.gpsimd.partition_all_reduce(
                amax_pr[:], amax1[:], channels=P, reduce_op=bass_isa.ReduceOp.max
            )

            # ---- global max across the 8 cores ----
            cin = dram.tile([P, 1], FP32)
            cout = dram.tile([P, 1], FP32, addr_space="Shared")
            nc.sync.dma_start(cin[:], amax_pr[:])
            nc.gpsimd.collective_compute(
                "AllReduce", mybir.AluOpType.max,
                replica_groups=[list(range(N_CORES))],
                ins=[cin[:].opt()], outs=[cout[:].opt()],
            )
            g_amax = const.tile([P, 1], FP32)
            nc.sync.dma_start(g_amax[:], cout[:])

            # ---- scale = where(amax > 0, amax/16, 1); inv = 1/scale ----
            m_t = const.tile([P, 1], FP32)
            nc.vector.tensor_scalar(m_t[:], g_amax[:], 0.0, None, mybir.AluOpType.is_gt)
            su = const.tile([P, 1], FP32)
            nc.vector.tensor_scalar(
                su[:], g_amax[:], 1.0 / 16.0, -1.0,
                mybir.AluOpType.mult, mybir.AluOpType.add,
            )
            nc.vector.tensor_tensor(su[:], su[:], m_t[:], mybir.AluOpType.mult)
            scale_t = const.tile([P, 1], FP32)
            nc.vector.tensor_scalar(scale_t[:], su[:], 1.0, None, mybir.AluOpType.add)
            inv_t = const.tile([P, 1], FP32)
            nc.vector.reciprocal(inv_t[:], scale_t[:])
            a_t = const.tile([P, 1], FP32)
            nc.vector.tensor_scalar(a_t[:], scale_t[:], 32.0 / 63.0, None, mybir.AluOpType.mult)
            c_t = const.tile([P, 1], FP32)
            nc.vector.tensor_scalar(c_t[:], scale_t[:], -16.0, None, mybir.AluOpType.mult)

            # ---- pass 2: dequantize into bf16 W.T SBUF cache ----
            # u = clip(W*inv, +-16); q = rne((u+16)*63/32); w = q*(32/63)*scale - 16*scale
            wt_sb = wt_pool.tile([P, KB, N], BF16)
            for kb in range(KB):
                wl = big.tile([P, N], FP32, tag="b2k")
                nc.sync.dma_start(wl[:], wT_d[kb * P : (kb + 1) * P, :])
                t = big.tile([P, N], FP32, tag="b2k")
                nc.vector.tensor_scalar(
                    t[:], wl[:], inv_t[:], 16.0,
                    mybir.AluOpType.mult, mybir.AluOpType.min,
                )
                nc.vector.tensor_scalar(
                    t[:], t[:], -16.0, 16.0,
                    mybir.AluOpType.max, mybir.AluOpType.add,
                )
                # round to nearest even via the 2^23 magic number
                nc.scalar.activation(
                    t[:], t[:], mybir.ActivationFunctionType.Copy,
                    scale=63.0 / 32.0, bias=8388608.0,
                )
                nc.vector.tensor_scalar(t[:], t[:], -8388608.0, None, mybir.AluOpType.add)
                nc.scalar.activation(
                    wt_sb[:, kb, :], t[:], mybir.ActivationFunctionType.Identity,
                    scale=a_t[:], bias=c_t[:],
                )

            # ---- main loop: y[mi] = x[mi] @ w_deq.T + bias ----
            xT_r = xT_d.rearrange("(b p) m -> p b m", p=P)  # [128, KB, M]
            for mi in range(MT):
                ms = mi * P
                xl0 = xload.tile([P, KB // 2, P], FP32, tag="xl")
                xl1 = xload.tile([P, KB // 2, P], FP32, tag="xl")
                nc.sync.dma_start(xl0[:], xT_r[:, 0 : KB // 2, ms : ms + P])
                nc.sync.dma_start(xl1[:], xT_r[:, KB // 2 : KB, ms : ms + P])
                xt_t = xt_pool.tile([P, KB, P], BF16)
                if mi % 2 == 0:
                    nc.vector.tensor_copy(xt_t[:, 0 : KB // 2, :], xl0[:])
                    nc.scalar.copy(xt_t[:, KB // 2 : KB, :], xl1[:])
                else:
                    nc.scalar.copy(xt_t[:, 0 : KB // 2, :], xl0[:])
                    nc.vector.tensor_copy(xt_t[:, KB // 2 : KB, :], xl1[:])

                ps = psum.tile([P, N], FP32)
                for kb in range(KB):
                    for nq in range(NQ):
                        nc.tensor.matmul(
                            ps[:, nq * NQS : (nq + 1) * NQS],
                            xt_t[:, kb, :],
                            wt_sb[:, kb, nq * NQS : (nq + 1) * NQS],
                            start=(kb == 0),
                            stop=(kb == KB - 1),
                        )
                ot = big.tile([P, N], FP32, tag="b2k")
                nc.vector.tensor_tensor(ot[:], ps[:], bias_rep[:], mybir.AluOpType.add)
                nc.sync.dma_start(y_d[ms : ms + P, :], ot[:])

    nc.compile()
    return nc


def _get_compiled():
    if "nc" not in _COMPILED:
        _COMPILED["nc"] = _build()
    return _COMPILED["nc"]


def kernel(x: np.ndarray, W: np.ndarray, bias: np.ndarray) -> np.ndarray:
    assert x.shape == (B, S, D_IN) and W.shape == (D_OUT, D_IN) and bias.shape == (D_OUT,)
    nc = _get_compiled()

    xT = np.ascontiguousarray(x.reshape(M, K).T.astype(np.float32, copy=False))
    in_maps = []
    for c in range(N_CORES):
        wT = np.ascontiguousarray(W[c * N : (c + 1) * N, :].T.astype(np.float32, copy=False))
        b = np.ascontiguousarray(bias[c * N : (c + 1) * N].astype(np.float32, copy=False)).reshape(1, N)
        in_maps.append({"xT": xT, "wT": wT, "bias": b})

    res = bass_utils.run_bass_kernel_spmd(nc, in_maps, core_ids=list(range(N_CORES)))
    y = np.concatenate([res.results[c]["y"] for c in range(N_CORES)], axis=1)
    return y.reshape(B, S, D_OUT)
